# revision 16
# baseline (speedup 1.0000x reference)
"""Masked dot-product attention on 8 Trainium2 NeuronCores.

Sharding: head-parallel. B*H = 64 (batch, head) pairs split 8 per core; each
core runs full attention for its heads.

Host-side staging (inside kernel(), before upload): Q and K are transposed to
[head, DK, S] and cast to bf16; V cast to bf16; the int32 mask is converted
once to a transposed bf16 "keep" matrix maskT[k, q] = 1 - mask[q, k]. This
leaves the device kernel with zero layout work: every PE cycle goes to the
QK^T and PV matmuls (plus the small output transposes).

Per-head-pair pipeline (S=2048, DK=64), in "S-transposed" layout so the PV
matmul consumes the exp matrix without transposing it:
  S_T[kj, qi] = K @ Q^T        (PE, bf16; the two heads of a pair run
                                concurrently on PE row groups 0-63 / 64-127)
  E_T = exp(S_T / sqrt(dk))    (ScalarE, PSUM -> SBUF bf16; no max-shift:
                                logits are ~N(0,1), exp cannot overflow, and
                                masked entries are zeroed after exp)
  E_T *= maskT (0/1 bf16)      (DVE, 2x mode, in-place)
  O_T[dv', qi] += V'[kj]^T E_T (PE accumulate over kj; V' has a ones column
                                so row dv'=64 accumulates the softmax denom)
  O = (O_T^T)[:, :64] * recip(O_T^T[:, 64])   (PE transpose + DVE)

The ScalarE exp stream is the roofline for this shape (~(N+352)/1.2 ns per
[128, N] activation, 1x rate, no accel): ~294us/core. Everything else is
scheduled to stay under it.
"""

import math

import numpy as np

import concourse.bass as bass
import concourse.mybir as mybir
import concourse.tile as tile
from concourse import bacc
from concourse.masks import make_identity

F32 = mybir.dt.float32
BF16 = mybir.dt.bfloat16
AF = mybir.ActivationFunctionType

N_CORES = 8


def build_attention_nc(nheads: int, S: int, DK: int, scale: float) -> bass.Bass:
    nc = bacc.Bacc("TRN2", target_bir_lowering=False, debug=False,
                   num_devices=N_CORES)

    q_d = nc.dram_tensor("queries", [nheads, DK, S], BF16, kind="ExternalInput")
    k_d = nc.dram_tensor("keys", [nheads, DK, S], BF16, kind="ExternalInput")
    v_d = nc.dram_tensor("values", [nheads, 128, S // 128, DK], BF16,
                         kind="ExternalInput")
    m_d = nc.dram_tensor("mask", [S, S], BF16, kind="ExternalInput")
    o_d = nc.dram_tensor("out", [nheads, S, DK], F32, kind="ExternalOutput")

    DV1 = DK + 1          # V plus a ones column for softmax denominators
    n_kj = S // 128       # kj tiles per head
    QBLK = min(512, S)    # qi span of one O_T accumulator
    n_qblk = S // QBLK
    OC = QBLK // 128      # 128-row output chunks per block
    CH = S // 128         # 128-row chunks along seq

    with tile.TileContext(nc) as tc:
        with (
            tc.tile_pool(name="consts", bufs=1) as consts,
            tc.tile_pool(name="maskT", bufs=1) as maskpool,
            tc.tile_pool(name="qkT", bufs=2) as qkt,
            tc.tile_pool(name="vp", bufs=2) as vp,
            tc.tile_pool(name="ep", bufs=16) as ep,
            tc.tile_pool(name="outp", bufs=4) as outp,
            tc.tile_pool(name="small", bufs=4) as small,
            tc.tile_pool(name="spsum", bufs=2, space="PSUM") as spsum,
            tc.tile_pool(name="opsum", bufs=1, space="PSUM") as opsum,
            tc.tile_pool(name="tpsum", bufs=2, space="PSUM") as tpsum,
        ):
            ident_f = consts.tile([DV1, DV1], F32)
            make_identity(nc, ident_f)

            maskT = [
                maskpool.tile([128, S], BF16, tag=f"maskT{kt}",
                              name=f"maskT_{kt}")
                for kt in range(n_kj)
            ]

            def load_pair(hp):
                # Q/K arrive pre-transposed [DK, S]; V pre-swizzled
                # [128, CH, DK] so every DMA reads per-partition-contiguous
                # lines (no descriptor spray).
                tiles = []
                for name, src in (("q", q_d), ("k", k_d)):
                    tT = qkt.tile([128, S], BF16, tag=f"{name}T",
                                  name=f"{name}T_{hp}")
                    tiles.append(tT)
                qT2, kT2 = tiles
                # h0 halves first: the first QK of the pair only needs rows
                # 0-63 of both tiles.
                for i in (0, 1):
                    nc.sync.dma_start(out=qT2[64 * i : 64 * i + 64, :],
                                      in_=q_d[2 * hp + i])
                    nc.sync.dma_start(out=kT2[64 * i : 64 * i + 64, :],
                                      in_=k_d[2 * hp + i])
                v1s = []
                for i in (0, 1):
                    v1 = vp.tile([128, CH, DV1], BF16, tag=f"v1_{i}",
                                 name=f"v1_{2 * hp + i}")
                    nc.gpsimd.dma_start(out=v1[:, :, 0:DK],
                                        in_=v_d[2 * hp + i])
                    nc.gpsimd.memset(v1[:, :, DK:DV1], 1.0)
                    v1s.append(v1)
                return qT2, kT2, v1s

            # Pair-0 load order is latency-critical: the first exp needs both
            # heads' QK, which needs only the first QBLK columns of qT/kT;
            # phase 0 needs only the first QBLK columns of every mask strip.
            # Load those slivers first, the bulk afterwards. gpsimd (SWDGE)
            # carries V and half the mask so the sync queue stays short.
            def load_pair0():
                tiles = []
                for name, src in (("q", q_d), ("k", k_d)):
                    tT = qkt.tile([128, S], BF16, tag=f"{name}T",
                                  name=f"{name}T_0")
                    tiles.append(tT)
                qT2, kT2 = tiles
                # kT is consumed across ALL columns in every phase (the kj
                # loop walks the whole key sequence), so it loads in full;
                # only qT is phase-local and can arrive in column slivers.
                for i in (0, 1):
                    nc.sync.dma_start(out=kT2[64 * i : 64 * i + 64, :],
                                      in_=k_d[i])
                    nc.sync.dma_start(
                        out=qT2[64 * i : 64 * i + 64, 0:QBLK],
                        in_=q_d[i][:, 0:QBLK],
                    )
                v1s = []
                for i in (0, 1):
                    v1 = vp.tile([128, CH, DV1], BF16, tag=f"v1_{i}",
                                 name=f"v1_{i}")
                    nc.gpsimd.dma_start(out=v1[:, :, 0:DK], in_=v_d[i])
                    nc.gpsimd.memset(v1[:, :, DK:DV1], 1.0)
                    v1s.append(v1)
                # first-QBLK slivers of every mask strip (gate phase 0)
                for kt in range(n_kj):
                    eng = nc.sync if kt % 2 == 0 else nc.gpsimd
                    eng.dma_start(
                        out=maskT[kt][:, 0:QBLK],
                        in_=m_d[kt * 128 : (kt + 1) * 128, 0:QBLK],
                    )
                # bulk: remaining qT columns, then remaining mask columns
                for i in (0, 1):
                    nc.sync.dma_start(
                        out=qT2[64 * i : 64 * i + 64, QBLK:S],
                        in_=q_d[i][:, QBLK:S],
                    )
                for kt in range(n_kj):
                    eng = nc.sync if kt % 2 == 1 else nc.gpsimd
                    eng.dma_start(
                        out=maskT[kt][:, QBLK:S],
                        in_=m_d[kt * 128 : (kt + 1) * 128, QBLK:S],
                    )
                return qT2, kT2, v1s

            pair_data = load_pair0()

            def emit_qk(qT2, kT2, hp, qb, kj):
                q0 = qb * QBLK
                ps_s = spsum.tile([128, 2 * QBLK], F32, tag="s",
                                  name=f"ps_s_{hp}_{qb}_{kj}")
                # high_priority keeps the h0/h1 pair adjacent in the PE
                # stream (they run concurrently on row groups); otherwise the
                # scheduler wedges stale PVs between them at phase
                # boundaries, which stalls the exp stream.
                with tc.high_priority(offset=12):
                    for i in (0, 1):
                        nc.tensor.matmul(
                            ps_s[:, i * QBLK : (i + 1) * QBLK],
                            kT2[64 * i : 64 * i + DK,
                                kj * 128 : (kj + 1) * 128],
                            qT2[64 * i : 64 * i + DK, q0 : q0 + QBLK],
                            start=True, stop=True,
                        )
                return ps_s

            def emit_exp_mask(ps_s, hp, qb, kj):
                q0 = qb * QBLK
                e_t = ep.tile([128, 2 * QBLK], BF16, tag="e",
                              name=f"e_{hp}_{qb}_{kj}")
                nc.scalar.activation(e_t, ps_s, AF.Exp, scale=scale)
                msl = maskT[kj][:, q0 : q0 + QBLK]
                mdup = bass.AP(
                    tensor=msl.tensor, offset=msl.offset,
                    ap=[msl.ap[0], [0, 2], msl.ap[-1]],
                )
                nc.vector.tensor_mul(e_t, e_t, mdup)
                return e_t

            def emit_pv(v1s, e_t, kj, ps_o):
                for i in (0, 1):
                    nc.tensor.matmul(
                        ps_o[:, i, :],
                        v1s[i][:, kj, :],
                        e_t[:, i * QBLK : (i + 1) * QBLK],
                        start=(kj == 0), stop=(kj == n_kj - 1),
                        skip_group_check=True,
                    )

            def emit_out_copy(hp, qb, ps_o):
                # One fused PSUM->SBUF copy frees both heads' accumulator
                # banks as early as possible (the next phase's PVs wait on
                # this buffer).
                ot_sb = outp.tile([DV1, 2, QBLK], F32, tag="ot",
                                  name=f"ot_{hp}_{qb}")
                nc.vector.tensor_copy(ot_sb, ps_o)
                return ot_sb

            def emit_out_finish(hp, qb, ot_sb):
                # Deferred into the next phase so the transposes/recip/mult
                # don't sit ahead of the next phase's mask-multiplies in the
                # DVE FIFO (or the QKs in the PE FIFO) at phase boundaries.
                q0 = qb * QBLK
                for i in (0, 1):
                    h = 2 * hp + i
                    ps_nat = tpsum.tile([128, OC, DV1], F32, tag="t",
                                        name=f"ps_nat_{h}_{qb}")
                    for c in range(OC):
                        nc.tensor.transpose(
                            ps_nat[:, c, :],
                            ot_sb[:, i, c * 128 : (c + 1) * 128],
                            ident_f,
                        )
                    rec = small.tile([128, OC], F32, tag="rec",
                                     name=f"rec_{h}_{qb}")
                    nc.vector.reciprocal(rec, ps_nat[:, :, DK])
                    o_sb = outp.tile([128, OC, DK], F32, tag="osb",
                                     name=f"o_sb_{h}_{qb}")
                    rb = bass.AP(tensor=rec.tensor, offset=rec.offset,
                                 ap=[rec.ap[0], rec.ap[-1], [0, DK]])
                    nc.vector.tensor_mul(o_sb, ps_nat[:, :, 0:DK], rb)
                    nc.sync.dma_start(
                        out=o_d[h, q0 : q0 + QBLK, :].rearrange(
                            "(c p) d -> p c d", p=128
                        ),
                        in_=o_sb,
                    )

            # Software-pipelined emission: the QK of step s+1 is enqueued on
            # the (strict-FIFO) PE queue before the PV of step s, so the PE
            # never blocks behind PV's wait for the DVE mask-multiply and the
            # ScalarE exp stream (the roofline) stays back-to-back.
            assert nheads % 2 == 0
            n_pairs = nheads // 2
            steps = [
                (hp, qb, kj)
                for hp in range(n_pairs)
                for qb in range(n_qblk)
                for kj in range(n_kj)
            ]

            pairs = {0: pair_data}
            ps_o_by_phase = {}

            def alloc_ps_o(hp, qb):
                ps_o_by_phase[(hp, qb)] = opsum.tile(
                    [DV1, 2, QBLK], F32, tag="o", name=f"ps_o_{hp}_{qb}"
                )

            alloc_ps_o(0, 0)
            qT2, kT2, v1s = pairs[0]
            ps_s_next = emit_qk(qT2, kT2, 0, 0, 0)
            pending_finish = None

            for s, (hp, qb, kj) in enumerate(steps):
                qT2, kT2, v1s = pairs[hp]
                if kj == 0 and qb == 0 and hp + 1 < n_pairs:
                    pairs[hp + 1] = load_pair(hp + 1)
                ps_s = ps_s_next
                e_t = emit_exp_mask(ps_s, hp, qb, kj)
                if s + 1 < len(steps):
                    nhp, nqb, nkj = steps[s + 1]
                    if (nhp, nqb) not in ps_o_by_phase:
                        alloc_ps_o(nhp, nqb)
                    nq, nk, _ = pairs[nhp]
                    ps_s_next = emit_qk(nq, nk, nhp, nqb, nkj)
                emit_pv(v1s, e_t, kj, ps_o_by_phase[(hp, qb)])
                if kj == 2 and pending_finish is not None:
                    emit_out_finish(*pending_finish)
                    pending_finish = None
                if kj == n_kj - 1:
                    ot_sb = emit_out_copy(hp, qb,
                                          ps_o_by_phase.pop((hp, qb)))
                    pending_finish = (hp, qb, ot_sb)
            emit_out_finish(*pending_finish)

    nc.compile()
    return nc


_NC_CACHE: dict = {}


def _get_nc(nheads, S, DK, scale):
    key = (nheads, S, DK, scale)
    if key not in _NC_CACHE:
        _NC_CACHE[key] = build_attention_nc(nheads, S, DK, scale)
    return _NC_CACHE[key]


def _prepare_in_maps(queries, keys, values, mask):
    """Host-side layout staging: transpose+cast Q/K, cast V, build the
    transposed bf16 keep-mask, and shard by head across the 8 cores."""
    import ml_dtypes

    bf16 = ml_dtypes.bfloat16
    B, H, S, DK = queries.shape
    BH = B * H
    hpc = BH // N_CORES

    qf = np.asarray(queries, dtype=np.float32).reshape(BH, S, DK)
    kf = np.asarray(keys, dtype=np.float32).reshape(BH, S, DK)
    vf = np.asarray(values, dtype=np.float32).reshape(BH, S, DK)
    qT = np.ascontiguousarray(qf.transpose(0, 2, 1)).astype(bf16)
    kT = np.ascontiguousarray(kf.transpose(0, 2, 1)).astype(bf16)
    # V swizzled to [head, partition, chunk, DK] so the device DMA reads
    # per-partition-contiguous 2KB lines.
    vb = np.ascontiguousarray(
        vf.reshape(BH, S // 128, 128, DK).transpose(0, 2, 1, 3)
    ).astype(bf16)
    mf = np.asarray(mask).reshape(S, S)
    mT = np.ascontiguousarray((1 - mf).T.astype(np.float32)).astype(bf16)

    return [
        {
            "queries": qT[c * hpc : (c + 1) * hpc],
            "keys": kT[c * hpc : (c + 1) * hpc],
            "values": vb[c * hpc : (c + 1) * hpc],
            "mask": mT,
        }
        for c in range(N_CORES)
    ]


def kernel(queries, keys, values, d_k, mask):
    from concourse.bass_utils import run_bass_kernel_spmd

    B, H, S, DK = queries.shape
    BH = B * H
    assert BH % N_CORES == 0
    hpc = BH // N_CORES
    scale = 1.0 / math.sqrt(float(d_k))

    nc = _get_nc(hpc, S, DK, scale)
    in_maps = _prepare_in_maps(queries, keys, values, mask)
    res = run_bass_kernel_spmd(nc, in_maps, core_ids=list(range(N_CORES)))
    out = np.concatenate([r["out"] for r in res.results], axis=0)
    return out.reshape(B, H, S, DK).astype(np.float32)
